# revision 1
# baseline (speedup 1.0000x reference)
"""Corr1d cost-volume kernel for Trainium2 (8 NeuronCores).

corr[b, d, h, x] = sum_c fL[b,c,h,x] * fR[b,c,h,x-d]  for x >= d, else 0.
Shapes: fL, fR = (4, 64, 256, 512) fp32; out = (4, 48, 256, 512) fp32.

Sharding: data-parallel over (batch, h-half): core i handles b = i//2,
h rows [128*(i%2), 128*(i%2)+128).

Per-core pipeline (per h row):
  - fp16 cast-load of fL/fR h-batches (SWDGE)
  - 4 banded matmuls (contract c=64 on partitions): lhsT = fL[c, x-block(128)],
    rhs = fR[c, window(176)] -> PSUM [128, 176] fp32
  - DVE copy PSUM -> SBUF fp16 data tile [128, 704]
  - gpsimd local_scatter with a constant per-partition index table: shears the
    diagonal band into a rect [128 x, 192 = 4 blocks x 48 d] (zero-filled)
  - 2 PE transposes [128, 96] -> PSUM [96, 128]
  - ACT copies -> fp32 assembly [96, NH*256]
  - 4 output DMAs per h-batch
"""
import numpy as np
from contextlib import ExitStack

import concourse.bass as bass
import concourse.tile as tile
import concourse.bacc as bacc
import concourse.mybir as mybir
from concourse import bass_utils
from concourse.ap import AP

B, C, H, W = 4, 64, 256, 512
D = 48
NCORES = 8
HH = H // 2            # h rows per core
NH = 16                # h rows per batch
NBATCH = HH // NH      # 16
WRHS = 192             # rhs window width (4 x 48 for the fold)
W0S = [0, 81, 209, 320]  # rhs window starts per x-block
NBLK = 4

fp16 = mybir.dt.float16
fp32 = mybir.dt.float32
i16 = mybir.dt.int16


def _make_tables():
    # band mask: mask[p, 192m + n] = 1 iff d = base_m + p - n in [0, 48)
    mask = np.zeros((128, NBLK * WRHS), dtype=np.float16)
    # rotation idx: folded col j of block m holds d = (base_m + p - j) mod 48
    idx1 = np.zeros((128, NBLK * D), dtype=np.int16)
    for m in range(NBLK):
        base = 128 * m - W0S[m]
        for p in range(128):
            for n in range(WRHS):
                if 0 <= base + p - n < D:
                    mask[p, WRHS * m + n] = 1.0
            for j in range(D):
                d = (base + p - j) % D
                idx1[p, D * m + j] = D * m + d
    parts = []
    for hi_ in range(NSC):
        t = idx1 + hi_ * NBLK * D
        parts.append(t)
    idx = np.concatenate(parts, axis=1)
    ident = np.eye(128, dtype=np.float16)
    return idx, mask, ident


NSC = 2                # h rows per local_scatter


def _build_nc():
    nc = bacc.Bacc("TRN2", target_bir_lowering=False, debug=False,
                   num_devices=NCORES)
    fL_d = nc.dram_tensor("fLc", [C, HH, W], fp16, kind="ExternalInput").ap()
    fR_d = nc.dram_tensor("fRc", [C, HH, W], fp16, kind="ExternalInput").ap()
    idx_d = nc.dram_tensor("idx", [128, NSC * NBLK * D], i16,
                           kind="ExternalInput").ap()
    mask_d = nc.dram_tensor("mask", [128, NBLK * WRHS], fp16,
                            kind="ExternalInput").ap()
    ident_d = nc.dram_tensor("ident", [128, 128], fp16, kind="ExternalInput").ap()
    out_d = nc.dram_tensor("outc", [D, HH, W], fp32, kind="ExternalOutput").ap()

    with tile.TileContext(nc) as tc, ExitStack() as ctx:
        const_pool = ctx.enter_context(tc.tile_pool(name="const", bufs=1))
        in_pool = ctx.enter_context(tc.tile_pool(name="inp", bufs=2))
        data_pool = ctx.enter_context(tc.tile_pool(name="data", bufs=6))
        band_pool = ctx.enter_context(tc.tile_pool(name="band", bufs=4))
        asm_pool = ctx.enter_context(tc.tile_pool(name="asm", bufs=2))
        mm_psum = ctx.enter_context(tc.tile_pool(name="mmps", bufs=5, space="PSUM"))
        tp_psum = ctx.enter_context(tc.tile_pool(name="tpps", bufs=3, space="PSUM"))

        idx_t = const_pool.tile([128, NSC * NBLK * D], i16)
        nc.sync.dma_start(idx_t[:], idx_d)
        mask_t = const_pool.tile([128, NBLK * WRHS], fp16)
        nc.sync.dma_start(mask_t[:], mask_d)
        ident_t = const_pool.tile([128, 128], fp16)
        nc.sync.dma_start(ident_t[:], ident_d)

        NHH = NH // 2  # h rows per partition-half
        # psum packing: 4 blocks of one h in 2 banks at these col offsets
        PS_OFF = [0, WRHS, 512, 512 + WRHS]
        for ib in range(NBATCH):
            h0 = ib * NH
            # h rows h0..h0+3 -> partitions 0:64, h0+4..h0+7 -> 64:128
            fl = in_pool.tile([128, NHH * W], fp16, tag="fl")
            fr = in_pool.tile([128, NHH * W], fp16, tag="fr")
            for half in range(2):
                nc.sync.dma_start(
                    fl[64 * half : 64 * half + 64, :]
                    .rearrange("c (h x) -> c h x", h=NHH),
                    fL_d[:, h0 + NHH * half : h0 + NHH * (half + 1), :],
                )
                nc.sync.dma_start(
                    fr[64 * half : 64 * half + 64, :]
                    .rearrange("c (h x) -> c h x", h=NHH),
                    fR_d[:, h0 + NHH * half : h0 + NHH * (half + 1), :],
                )

            asm = asm_pool.tile([96, NH * 256], fp32)

            def emit_transposes(band_, hp_):
                tp = tp_psum.tile([96, 512], fp16)
                for tt in range(4):
                    nc.tensor.transpose(
                        tp[:, 128 * tt : 128 * tt + 128],
                        band_[:, 96 * tt : 96 * tt + 96], ident_t[:]
                    )
                nc.scalar.copy(
                    asm[:].rearrange("q (hh x) -> q hh x", hh=NH)[:, hp_::NHH, :],
                    tp[:].rearrange("q (hb x) -> q hb x", hb=2),
                )

            pending = None
            for hp in range(NHH):
                # pair (hA, hB) = (hp, hp + NHH): hA on partitions 0:64,
                # hB on 64:128; PE row-group concurrency per block.
                # one PSUM bank per (hi, block-pair): [128, 512] holds 2 blocks
                pss = []
                for hi in range(2):
                    row = []
                    for bk in range(2):
                        ps = mm_psum.tile([128, 512], fp32, tag="mmps")
                        row.append(ps)
                    pss.append(row)
                for m in range(NBLK):
                    for hi in range(2):
                        pb = 64 * hi
                        nc.tensor.matmul(
                            pss[hi][m // 2][:, WRHS * (m % 2) :
                                            WRHS * (m % 2) + WRHS],
                            fl[pb : pb + 64,
                               hp * W + 128 * m : hp * W + 128 * m + 128],
                            fr[pb : pb + 64,
                               hp * W + W0S[m] : hp * W + W0S[m] + WRHS],
                            start=True,
                            stop=True,
                        )
                folded = data_pool.tile([128, 2 * NBLK * D], fp16, tag="folded")
                for hi in range(2):
                    # masked evacuation in k-major layout: psum col
                    # (m, 48k + j) -> data col 192k + 48m + j, so the fold
                    # adds below are flat contiguous halves.
                    data = data_pool.tile([128, NBLK * WRHS], fp16, tag="data")
                    for bk in range(2):
                        nc.vector.tensor_mul(
                            data[:]
                            .rearrange("p (k m j) -> p k m j", k=4, m=NBLK)
                            [:, :, 2 * bk : 2 * bk + 2, :]
                            .transpose([0, 2, 1, 3]),
                            pss[hi][bk][:, 0 : 2 * WRHS]
                            .rearrange("p (m k j) -> p m k j", m=2, k=4),
                            mask_t[:, 2 * WRHS * bk : 2 * WRHS * (bk + 1)]
                            .rearrange("p (m k j) -> p m k j", m=2, k=4),
                        )
                    # fold: sum the 4 k-planes (flat contiguous adds)
                    t1 = data_pool.tile([128, NBLK * 96], fp16, tag="t1")
                    with nc.allow_low_precision(reason="fold adds zeros"):
                        nc.vector.tensor_add(
                            t1[:], data[:, 0:384], data[:, 384:768]
                        )
                        nc.vector.tensor_add(
                            folded[:, NBLK * D * hi : NBLK * D * (hi + 1)],
                            t1[:, 0:192], t1[:, 192:384],
                        )
                band = band_pool.tile([128, 2 * NBLK * D], fp16)
                nc.gpsimd.local_scatter(
                    band[:], folded[:], idx_t[:],
                    channels=128, num_elems=2 * NBLK * D,
                    num_idxs=2 * NBLK * D,
                )
                tp = tp_psum.tile([96, 512], fp16)
                for tt in range(4):
                    nc.tensor.transpose(
                        tp[:, 128 * tt : 128 * tt + 128],
                        band[:, 96 * tt : 96 * tt + 96], ident_t[:]
                    )
                nc.scalar.copy(
                    asm[:].rearrange("q (hh x) -> q hh x", hh=NH)[:, hp::NHH, :],
                    tp[:].rearrange("q (hb x) -> q hb x", hb=2),
                )

            # output DMAs: asm[48*par + d, h*256 + t*128 + xin] ->
            #   out[d, h0+h, 256t + 128par + xin]
            for par in range(2):
                for t in range(2):
                    nc.sync.dma_start(
                        out_d[:, h0 : h0 + NH, 256 * t + 128 * par :
                              256 * t + 128 * par + 128],
                        asm[48 * par : 48 * par + 48, :]
                        .rearrange("d (h x) -> d h x", h=NH)[:, :, 128 * t : 128 * t + 128],
                    )

    nc.compile()
    return nc


_NC_CACHE = None


def _get_nc():
    global _NC_CACHE
    if _NC_CACHE is None:
        _NC_CACHE = _build_nc()
    return _NC_CACHE


def kernel(fL: np.ndarray, fR: np.ndarray) -> np.ndarray:
    fL = np.asarray(fL, dtype=np.float32)
    fR = np.asarray(fR, dtype=np.float32)
    nc = _get_nc()
    idx, mask, ident = _make_tables()

    in_maps = []
    for core in range(NCORES):
        b, half = divmod(core, 2)
        sl = np.s_[b, :, half * HH : half * HH + HH, :]
        in_maps.append({
            "fLc": fL[sl].astype(np.float16),
            "fRc": fR[sl].astype(np.float16),
            "idx": idx,
            "mask": mask,
            "ident": ident,
        })

    res = bass_utils.run_bass_kernel_spmd(nc, in_maps, core_ids=list(range(NCORES)))
    out = np.empty((B, D, H, W), dtype=np.float32)
    for core in range(NCORES):
        b, half = divmod(core, 2)
        out[b, :, half * HH : half * HH + HH, :] = res.results[core]["outc"]
    return out



# revision 3
# speedup vs baseline: 1.5557x; 1.5557x over previous
"""Corr1d cost-volume kernel for Trainium2 (8 NeuronCores).

corr[b, d, h, x] = sum_c fL[b,c,h,x] * fR[b,c,h,x-d]  for x >= d, else 0.
Shapes: fL, fR = (4, 64, 256, 512) fp32; out = (4, 48, 256, 512) fp32.

Sharding: data-parallel over (batch, h-half): core i handles b = i//2,
h rows [128*(i%2), 128*(i%2)+128).

Per-core pipeline, per hp (2 h rows, hi=0/1 on c-partition halves):
  - 8 banded matmuls (contract c=64): lhsT = fL[c, 128-x-block],
    rhs = zero-padded fR window (192 cols starting at x' = 128m-47)
    -> PSUM windows at 256-col alignment (no bank crossing)
  - DVE (hi=0) / ACT (hi=1) plain copy PSUM -> SBUF fp16 stage (no masks:
    the 48-col zero pad in fR makes the x<d region exactly 0)
  - one contiguous [128, 1536] fp16 DMA stage -> DRAM (full windows)
Host extracts the band: out[d, h, 128m+q] = win[.., q, hi, m, c=q+47-d]
via a take_along_axis gather, then casts fp32.
"""
import numpy as np
from contextlib import ExitStack

import concourse.bass as bass
import concourse.tile as tile
import concourse.bacc as bacc
import concourse.mybir as mybir
from concourse import bass_utils

B, C, H, W = 4, 64, 256, 512
D = 48
NCORES = 8
HH = H // 2           # h rows per core
NB = 8                # batches per core
NH = 16               # h rows per batch
NHH = 8               # hp iterations per batch (2 rows each)
PAD = 48              # left zero pad on fR rows
WP = W + PAD          # padded fR row width
WIN = 192             # window width
NWIN = 4              # x-blocks of 128 per row
STG = 2 * NWIN * WIN  # 1536 stage cols

fp16 = mybir.dt.float16
fp32 = mybir.dt.float32


def _build_nc():
    nc = bacc.Bacc("TRN2", target_bir_lowering=False, debug=False,
                   num_devices=NCORES)
    fL_d = nc.dram_tensor("fLc", [NB, 128, NHH * W], fp16,
                          kind="ExternalInput").ap()
    fR_d = nc.dram_tensor("fRc", [NB, 128, NHH * WP], fp16,
                          kind="ExternalInput").ap()
    out_d = nc.dram_tensor("outc", [NB, NHH, 128, STG], fp16,
                           kind="ExternalOutput").ap()

    with tile.TileContext(nc) as tc, ExitStack() as ctx:
        in_pool = ctx.enter_context(tc.tile_pool(name="inp", bufs=2))
        stg_pool = ctx.enter_context(tc.tile_pool(name="stg", bufs=3))
        ps_pool = ctx.enter_context(tc.tile_pool(name="ps", bufs=4,
                                                 space="PSUM"))

        for ib in range(NB):
            fl = in_pool.tile([128, NHH * W], fp16, tag="fl")
            fr = in_pool.tile([128, NHH * WP], fp16, tag="fr")
            nc.sync.dma_start(fl[:], fL_d[ib])
            nc.sync.dma_start(fr[:], fR_d[ib])

            for hp in range(NHH):
                pss = []
                for hi in range(2):
                    ps = ps_pool.tile([128, 1024], fp32, tag="ps")
                    pb = 64 * hi
                    for m in range(NWIN):
                        n = WIN if m < 3 else 175  # m=3 stops at row end
                        nc.tensor.matmul(
                            ps[:, 256 * m : 256 * m + n],
                            fl[pb : pb + 64,
                               hp * W + 128 * m : hp * W + 128 * m + 128],
                            fr[pb : pb + 64,
                               hp * WP + 128 * m + 1 :
                               hp * WP + 128 * m + 1 + n],
                            start=True, stop=True,
                        )
                    pss.append(ps)

                stage = stg_pool.tile([128, STG], fp16)
                nc.vector.tensor_copy(
                    stage[:, 0 : NWIN * WIN]
                    .rearrange("p (m c) -> p m c", m=NWIN),
                    pss[0][:].rearrange("p (m c) -> p m c", m=NWIN)
                    [:, :, 0:WIN],
                )
                nc.scalar.copy(
                    stage[:, NWIN * WIN : STG]
                    .rearrange("p (m c) -> p m c", m=NWIN),
                    pss[1][:].rearrange("p (m c) -> p m c", m=NWIN)
                    [:, :, 0:WIN],
                )
                eng = nc.sync if hp % 2 == 0 else nc.scalar
                eng.dma_start(out_d[ib, hp], stage[:])

    nc.compile()
    return nc


_NC_CACHE = None


def _get_nc():
    global _NC_CACHE
    if _NC_CACHE is None:
        _NC_CACHE = _build_nc()
    return _NC_CACHE


def _prep_core_inputs(fL, fR, core):
    b, half = divmod(core, 2)
    sl = np.s_[b, :, half * HH : half * HH + HH, :]
    fLs = fL[sl].astype(np.float16)                     # (64, 128, 512)
    fRs = fR[sl].astype(np.float16)
    frp = np.zeros((C, HH, WP), dtype=np.float16)
    frp[:, :, PAD:] = fRs
    # h = 16*ib + 8*hi + hp  ->  reshape h as (ib, hi, hp)
    fLc = (fLs.reshape(C, NB, 2, NHH, W)
           .transpose(1, 2, 0, 3, 4).reshape(NB, 128, NHH * W))
    fRc = (frp.reshape(C, NB, 2, NHH, WP)
           .transpose(1, 2, 0, 3, 4).reshape(NB, 128, NHH * WP))
    return np.ascontiguousarray(fLc), np.ascontiguousarray(fRc)


# c index for band extraction: c = q + 47 - d   (q: x within 128-block)
_CIDX = (np.arange(128)[:, None] + (D - 1) - np.arange(D)[None, :])  # (128,48)


def kernel(fL: np.ndarray, fR: np.ndarray) -> np.ndarray:
    fL = np.asarray(fL, dtype=np.float32)
    fR = np.asarray(fR, dtype=np.float32)
    nc = _get_nc()

    in_maps = []
    for core in range(NCORES):
        fLc, fRc = _prep_core_inputs(fL, fR, core)
        in_maps.append({"fLc": fLc, "fRc": fRc})

    res = bass_utils.run_bass_kernel_spmd(nc, in_maps,
                                          core_ids=list(range(NCORES)))
    out = np.empty((B, D, H, W), dtype=np.float32)
    cidx = _CIDX[None, None, None, None]                # (1,1,1,1,128,48)
    for core in range(NCORES):
        b, half = divmod(core, 2)
        win = res.results[core]["outc"].reshape(NB, NHH, 128, 2, NWIN, WIN)
        # -> (ib, hi, hp, m, q, c)
        wt = win.transpose(0, 3, 1, 4, 2, 5)
        band = np.take_along_axis(wt, cidx, axis=-1)    # (ib,hi,hp,m,q,48)
        # -> (d, ib, hi, hp, m, q) -> (48, 128, 512)
        oc = band.transpose(5, 0, 1, 2, 3, 4).reshape(D, HH, W)
        out[b, :, half * HH : half * HH + HH, :] = oc.astype(np.float32)
    return out
